# revision 12
# baseline (speedup 1.0000x reference)
"""CentroidPool (knn argmin) Trainium2 kernel.

kernel(latent [131072,128] f32, coords [1024,128] f32) -> closest-centroid
index per row, int32 [131072].

Data-parallel over rows across 8 NeuronCores. Host sorts the 1024 centroids
by |c|^2 and lays rank 2j at score column j, rank 2j+1 at column j+512, so
column pairs (j, j+512) are c2-adjacent. Device computes raw scores
u = 2*x@c.T as bf16 matmuls into PSUM f32 (two 512-wide MMs per 128-row
tile; tiles processed in pairs sharing one 4-bank PSUM tile).

The PSUM drain is the bottleneck (ScalarE ~0.94 f32/ns/partition, VectorE
reduce ~0.71, both measured), so tile-pairs are statically assigned one of
three drain modes, Bresenham-interleaved:
  R: ScalarE converts the 2048-wide block to fp16; raw scores DMA out.
     Host does the argmax at per-centroid resolution (tightest pruning).
  F: ScalarE converts; VectorE folds column halves in 2x 16-bit mode
     (pair-maxes out; half the DMA of R). Soaks spare VectorE cycles.
  V: VectorE max-reduces pairs (j, j+512) straight from PSUM via a strided
     view; ScalarE untouched.
Host brackets each centroid (R) or pair (F/V) score with a bf16+fp16 noise
margin, prunes, and resolves the few candidates exactly in fp64 with
first-index tie-breaking.
"""

from contextlib import ExitStack

import numpy as np
import ml_dtypes

import concourse.bacc as bacc
import concourse.mybir as mybir
import concourse.tile as tile
from concourse.bass_utils import run_bass_kernel_spmd

N = 131072
D = 128
K = 1024
N_CORES = 8
ROWS_PER_CORE = N // N_CORES        # 16384
TILE_ROWS = 128
N_TILES = ROWS_PER_CORE // TILE_ROWS  # 128
N_PAIRS = N_TILES // 2                # 64
CHUNK_TILES = 8
NP = K // 2                          # 512 column pairs
MARGIN_RAW = 2.0                     # 2x (bf16 matmul + fp16 round) + slack
MARGIN_PAIR = 2.5
QUOTA_R, QUOTA_F = 67, 0             # of 128 tile-units; rest are V

F32 = mybir.dt.float32
BF16 = mybir.dt.bfloat16
FP16 = mybir.dt.float16

_CACHE: dict = {}


def _pattern(n_units: int = N_TILES):
    """Bresenham-interleave R/V modes at QUOTA_R/(rest) per 128 tile-units."""
    quotas = {"R": QUOTA_R, "V": N_TILES - QUOTA_R}
    acc = {m: 0 for m in quotas}
    out = []
    for _ in range(n_units):
        for m in quotas:
            acc[m] += quotas[m]
        pick = max(acc, key=lambda m: (acc[m], quotas[m]))
        acc[pick] -= N_TILES
        out.append(pick)
    return out


def _build_program(n_tiles: int = N_TILES, reps: int = 1,
                   chunk_tiles: int = CHUNK_TILES,
                   psum_bufs: int = 4, sh_bufs: int = 3,
                   vout_bufs: int = 3, lchunk_bufs: int = 3):
    nc = bacc.Bacc("TRN2", target_bir_lowering=False, debug=False,
                   num_devices=N_CORES)
    n_rows = n_tiles * TILE_ROWS
    CHT = chunk_tiles
    pat = _pattern(n_tiles)
    n_r = sum(m == "R" for m in pat)
    n_v = sum(m == "V" for m in pat)

    lat_t = nc.dram_tensor("lat_t", [D, n_rows], BF16, kind="ExternalInput").ap()
    c2t = nc.dram_tensor("c2t", [D, K], BF16, kind="ExternalInput").ap()
    gm_raw = nc.dram_tensor("gm_raw", [TILE_ROWS, max(n_r, 1) * K], FP16,
                            kind="ExternalOutput").ap()
    gm_vred = nc.dram_tensor("gm_vred", [TILE_ROWS, max(n_v, 1) * NP], FP16,
                             kind="ExternalOutput").ap()

    with ExitStack() as ctx:
        tc = ctx.enter_context(tile.TileContext(nc))
        const_pool = ctx.enter_context(tc.tile_pool(name="const", bufs=1))
        lchunk_pool = ctx.enter_context(tc.tile_pool(name="lchunk",
                                                     bufs=lchunk_bufs))
        psum_pool = ctx.enter_context(tc.tile_pool(name="psum", bufs=psum_bufs,
                                                   space="PSUM"))
        sh_pool = ctx.enter_context(tc.tile_pool(name="sh", bufs=sh_bufs))
        vout_pool = ctx.enter_context(tc.tile_pool(name="vout",
                                                   bufs=vout_bufs))

        c2t_sb = const_pool.tile([D, K], BF16)
        nc.sync.dma_start(c2t_sb[:], c2t[:])

        assert n_tiles % CHT == 0

        def body():
            o_r = o_v = 0
            for c in range(n_tiles // CHT):
                t0 = c * CHT
                k_r = sum(pat[t0 + u] == "R" for u in range(CHT))
                k_v = CHT - k_r
                lchunk = lchunk_pool.tile([D, CHT * TILE_ROWS], BF16,
                                          tag="lchunk")
                nc.sync.dma_start(
                    lchunk[:], lat_t[:, t0 * TILE_ROWS:(t0 + CHT) * TILE_ROWS])
                stg_r = sh_pool.tile([TILE_ROWS, CHT * K], FP16, tag="sh")
                stg_v = vout_pool.tile([TILE_ROWS, CHT * NP], FP16, tag="vo")
                cr = cv = 0
                for u in range(CHT):
                    mode = pat[t0 + u]
                    ps = psum_pool.tile([TILE_ROWS, K], F32, tag="ps")
                    lt = lchunk[:, u * TILE_ROWS:(u + 1) * TILE_ROWS]
                    for h in range(2):
                        nc.tensor.matmul(
                            ps[:, h * 512:(h + 1) * 512],
                            lt, c2t_sb[:, h * 512:(h + 1) * 512],
                            start=True, stop=True)
                    if mode == "R":
                        # ScalarE drains the whole unit to fp16 staging.
                        nc.scalar.copy(stg_r[:, cr * K:(cr + 1) * K], ps[:])
                        cr += 1
                    else:
                        # VectorE max-reduces pairs (j, j+512) from PSUM.
                        nc.vector.tensor_reduce(
                            out=stg_v[:, cv * NP:(cv + 1) * NP],
                            in_=ps[:].rearrange("p (l j) -> p j l", l=2),
                            axis=mybir.AxisListType.X,
                            op=mybir.AluOpType.max)
                        cv += 1
                # one consolidated output DMA per stream per chunk
                if k_r:
                    nc.sync.dma_start(gm_raw[:, o_r * K:(o_r + k_r) * K],
                                      stg_r[:, 0:k_r * K])
                if k_v:
                    nc.sync.dma_start(gm_vred[:, o_v * NP:(o_v + k_v) * NP],
                                      stg_v[:, 0:k_v * NP])
                o_r += k_r
                o_v += k_v

        if reps == 1:
            body()
        else:
            with tc.For_i(0, reps, 1):
                body()

    nc.compile()
    return nc


def _get_program():
    if "nc" not in _CACHE:
        _CACHE["nc"] = _build_program()
    return _CACHE["nc"]


def _centroid_perm(coords: np.ndarray):
    """Column layout: col j = c2-rank 2j, col j+512 = rank 2j+1."""
    c2_64 = (coords.astype(np.float64) ** 2).sum(1)
    order = np.argsort(c2_64, kind="stable").astype(np.int64)
    cols = np.empty(K, np.int64)
    cols[:NP] = order[0::2]
    cols[NP:] = order[1::2]
    return c2_64, order, cols


def make_in_maps(latent: np.ndarray, coords: np.ndarray) -> list[dict]:
    _, _, cols = _centroid_perm(coords)
    c2t = np.ascontiguousarray(
        (2.0 * coords[cols].T).astype(ml_dtypes.bfloat16))
    in_maps = []
    for c in range(N_CORES):
        sl = slice(c * ROWS_PER_CORE, (c + 1) * ROWS_PER_CORE)
        in_maps.append({
            "lat_t": np.ascontiguousarray(
                latent[sl].T.astype(ml_dtypes.bfloat16)),
            "c2t": c2t,
        })
    return in_maps


def kernel(latent: np.ndarray, coords: np.ndarray) -> np.ndarray:
    latent = np.asarray(latent, dtype=np.float32)
    coords = np.asarray(coords, dtype=np.float32)
    assert latent.shape == (N, D) and coords.shape == (K, D)

    nc = _get_program()
    in_maps = make_in_maps(latent, coords)
    res = run_bass_kernel_spmd(nc, in_maps, list(range(N_CORES)))

    c2_64, order, cols = _centroid_perm(coords)
    pat = _pattern()

    # Reassemble per-row score arrays. Raw tiles get u at column resolution
    # [m, 1024] (column order = cols); vred tiles get pair maxes [m, 512]
    # for pairs (rank 2j, rank 2j+1).
    raw_rows, raw_u = [], []
    pair_rows, pair_m = [], []
    for c in range(N_CORES):
        r = res.results[c]
        raws = r["gm_raw"].reshape(TILE_ROWS, -1, K)      # [p, ord, col]
        vreds = r["gm_vred"].reshape(TILE_ROWS, -1, NP)
        ords = {"R": 0, "V": 0}
        for t, mode in enumerate(pat):
            o = ords[mode]
            ords[mode] += 1
            rows = (c * ROWS_PER_CORE + t * TILE_ROWS + np.arange(TILE_ROWS))
            if mode == "R":
                raw_rows.append(rows)
                raw_u.append(raws[:, o, :])
            else:
                pair_rows.append(rows)
                pair_m.append(vreds[:, o, :])
    raw_rows = np.concatenate(raw_rows)
    raw_u = np.concatenate(raw_u).astype(np.float32)
    pair_rows = np.concatenate(pair_rows)
    pair_m = np.concatenate(pair_m).astype(np.float32)

    out = np.empty(N, np.int64)
    lat64 = latent.astype(np.float64)
    coords64 = coords.astype(np.float64)
    c2s = c2_64[order]

    # --- raw rows: per-centroid bracket ---------------------------------
    c2_cols = c2_64[cols].astype(np.float32)
    s_est = raw_u - c2_cols[None, :]
    best = s_est.max(1)
    n_cand = (s_est >= best[:, None] - MARGIN_RAW).sum(1)
    CMAXR = 4
    _resolve(out, raw_rows, s_est, n_cand, CMAXR, lat64, coords64, c2_64,
             cols.reshape(K, 1), coords64[cols].reshape(K, 1, D),
             c2_64[cols].reshape(K, 1), MARGIN_RAW)

    # --- fold/vred rows: pair bracket -----------------------------------
    c2min = c2s[0::2].astype(np.float32)
    c2max = c2s[1::2].astype(np.float32)
    ub = pair_m - c2min[None, :]
    lb = pair_m - c2max[None, :]
    best_lb = lb.max(1)
    n_cand = (ub >= best_lb[:, None] - MARGIN_PAIR).sum(1)
    CMAXP = 8
    _resolve(out, pair_rows, ub, n_cand, CMAXP, lat64, coords64, c2_64,
             order.reshape(NP, 2), coords64[order].reshape(NP, 2, D),
             c2s.reshape(NP, 2), MARGIN_PAIR)

    return out.astype(np.int32)


def _resolve(out, rows, ub, n_cand, cmax, lat64, coords64, c2, group_idx,
             group_c, group_c2, margin):
    """Resolve rows' argmin: bulk rows use top-cmax groups by ub (superset of
    candidates when n_cand <= cmax), rest fall back to the full fp64 sweep.
    First-original-index tie-breaking throughout."""
    L = group_idx.shape[1]
    bulk_m = n_cand <= cmax
    bulk = rows[bulk_m]
    if bulk.size:
        ubb = ub[bulk_m]
        gsel = np.argpartition(-ubb, cmax - 1, axis=1)[:, :cmax]
        m = bulk.size
        cands = group_c[gsel]                     # [m, C, L, D]
        sc = 2.0 * np.einsum('md,mcld->mcl', lat64[bulk], cands,
                             optimize=True) - group_c2[gsel]
        sc = sc.reshape(m, cmax * L)
        orig = group_idx[gsel].reshape(m, cmax * L)
        best = sc.max(1)
        is_best = sc >= best[:, None]
        masked = np.where(is_best, orig, np.int64(1 << 60))
        out[bulk] = masked.min(1)
    rest = rows[~bulk_m]
    if rest.size:
        sc = 2.0 * lat64[rest] @ coords64.T - c2[None, :]
        best = sc.max(1)
        is_best = sc >= best[:, None]
        masked = np.where(is_best, np.arange(len(c2))[None, :],
                          np.int64(1 << 60))
        out[rest] = masked.min(1)


# revision 13
# speedup vs baseline: 1.1234x; 1.1234x over previous
"""CentroidPool (knn argmin) Trainium2 kernel.

kernel(latent [131072,128] f32, coords [1024,128] f32) -> closest-centroid
index per row, int32 [131072].

Data-parallel over rows across 8 NeuronCores. Host sorts the 1024 centroids
by |c|^2 and lays rank 2j at score column j, rank 2j+1 at column j+512, so
column pairs (j, j+512) are c2-adjacent. Device computes raw scores
u = 2*x@c.T as bf16 matmuls into PSUM f32, one 128-row x 1024-col unit per
tile (psum_bufs=4 keeps PE stalls fine-grained so the HAM clock stays up).

The PSUM drain is the bottleneck (measured: ScalarE copy 1024 f32->fp16
~1.33us, VectorE strided pair-reduce ~1.44us, vs PE fill ~0.9us), so tiles
are statically assigned one of two drain modes, Bresenham-interleaved at
67:61 to balance the engines:
  R: ScalarE converts the unit to fp16 staging; raw scores DMA out. Host
     does the argmax at per-centroid resolution (tightest pruning).
  V: VectorE max-reduces pairs (j, j+512) straight from PSUM via a strided
     view; ScalarE untouched.
Output DMAs are consolidated per 8-tile chunk (per-unit dma_starts cost
~40us of sync-engine issue overhead; measured 131->103us from this alone).
Host brackets each centroid (R) or pair (V) score with a bf16+fp16 noise
margin, prunes, and resolves the few candidates exactly in fp64 with
first-index tie-breaking.

History: 132.6us baseline (all-ScalarE drain, fp32r) -> 103.1us measured
(engine-balanced drain, bf16 matmuls, chunked DMA). Engine budget/core:
SE 89us, VE 88us, DMA 82us (346GB/s measured), PE 61us warm / 113us cold.
"""

from contextlib import ExitStack

import numpy as np
import ml_dtypes

import concourse.bacc as bacc
import concourse.mybir as mybir
import concourse.tile as tile
from concourse.bass_utils import run_bass_kernel_spmd

N = 131072
D = 128
K = 1024
N_CORES = 8
ROWS_PER_CORE = N // N_CORES        # 16384
TILE_ROWS = 128
N_TILES = ROWS_PER_CORE // TILE_ROWS  # 128
N_PAIRS = N_TILES // 2                # 64
CHUNK_TILES = 8
NP = K // 2                          # 512 column pairs
MARGIN_RAW = 2.0                     # 2x (bf16 matmul + fp16 round) + slack
MARGIN_PAIR = 2.5
QUOTA_R, QUOTA_F = 67, 0             # of 128 tile-units; rest are V

F32 = mybir.dt.float32
BF16 = mybir.dt.bfloat16
FP16 = mybir.dt.float16

_CACHE: dict = {}


def _pattern(n_units: int = N_TILES):
    """Bresenham-interleave R/V modes at QUOTA_R/(rest) per 128 tile-units."""
    quotas = {"R": QUOTA_R, "V": N_TILES - QUOTA_R}
    acc = {m: 0 for m in quotas}
    out = []
    for _ in range(n_units):
        for m in quotas:
            acc[m] += quotas[m]
        pick = max(acc, key=lambda m: (acc[m], quotas[m]))
        acc[pick] -= N_TILES
        out.append(pick)
    return out


def _build_program(n_tiles: int = N_TILES, reps: int = 1,
                   chunk_tiles: int = CHUNK_TILES,
                   psum_bufs: int = 4, sh_bufs: int = 2,
                   vout_bufs: int = 2, lchunk_bufs: int = 3):
    nc = bacc.Bacc("TRN2", target_bir_lowering=False, debug=False,
                   num_devices=N_CORES)
    n_rows = n_tiles * TILE_ROWS
    CHT = chunk_tiles
    pat = _pattern(n_tiles)
    n_r = sum(m == "R" for m in pat)
    n_v = sum(m == "V" for m in pat)

    lat_t = nc.dram_tensor("lat_t", [D, n_rows], BF16, kind="ExternalInput").ap()
    c2t = nc.dram_tensor("c2t", [D, K], BF16, kind="ExternalInput").ap()
    gm_raw = nc.dram_tensor("gm_raw", [TILE_ROWS, max(n_r, 1) * K], FP16,
                            kind="ExternalOutput").ap()
    gm_vred = nc.dram_tensor("gm_vred", [TILE_ROWS, max(n_v, 1) * NP], FP16,
                             kind="ExternalOutput").ap()

    with ExitStack() as ctx:
        tc = ctx.enter_context(tile.TileContext(nc))
        const_pool = ctx.enter_context(tc.tile_pool(name="const", bufs=1))
        lchunk_pool = ctx.enter_context(tc.tile_pool(name="lchunk",
                                                     bufs=lchunk_bufs))
        psum_pool = ctx.enter_context(tc.tile_pool(name="psum", bufs=psum_bufs,
                                                   space="PSUM"))
        sh_pool = ctx.enter_context(tc.tile_pool(name="sh", bufs=sh_bufs))
        vout_pool = ctx.enter_context(tc.tile_pool(name="vout",
                                                   bufs=vout_bufs))

        c2t_sb = const_pool.tile([D, K], BF16)
        nc.sync.dma_start(c2t_sb[:], c2t[:])

        assert n_tiles % CHT == 0

        def body():
            o_r = o_v = 0
            for c in range(n_tiles // CHT):
                t0 = c * CHT
                k_r = sum(pat[t0 + u] == "R" for u in range(CHT))
                k_v = CHT - k_r
                lchunk = lchunk_pool.tile([D, CHT * TILE_ROWS], BF16,
                                          tag="lchunk")
                nc.sync.dma_start(
                    lchunk[:], lat_t[:, t0 * TILE_ROWS:(t0 + CHT) * TILE_ROWS])
                stg_r = sh_pool.tile([TILE_ROWS, CHT * K], FP16, tag="sh")
                stg_v = vout_pool.tile([TILE_ROWS, CHT * NP], FP16, tag="vo")
                cr = cv = 0
                for u in range(CHT):
                    mode = pat[t0 + u]
                    ps = psum_pool.tile([TILE_ROWS, K], F32, tag="ps")
                    lt = lchunk[:, u * TILE_ROWS:(u + 1) * TILE_ROWS]
                    for h in range(2):
                        nc.tensor.matmul(
                            ps[:, h * 512:(h + 1) * 512],
                            lt, c2t_sb[:, h * 512:(h + 1) * 512],
                            start=True, stop=True)
                    if mode == "R":
                        # ScalarE drains the whole unit to fp16 staging.
                        nc.scalar.copy(stg_r[:, cr * K:(cr + 1) * K], ps[:])
                        cr += 1
                    else:
                        # VectorE max-reduces pairs (j, j+512) from PSUM.
                        nc.vector.tensor_reduce(
                            out=stg_v[:, cv * NP:(cv + 1) * NP],
                            in_=ps[:].rearrange("p (l j) -> p j l", l=2),
                            axis=mybir.AxisListType.X,
                            op=mybir.AluOpType.max)
                        cv += 1
                # one consolidated output DMA per stream per chunk
                if k_r:
                    nc.sync.dma_start(gm_raw[:, o_r * K:(o_r + k_r) * K],
                                      stg_r[:, 0:k_r * K])
                if k_v:
                    nc.sync.dma_start(gm_vred[:, o_v * NP:(o_v + k_v) * NP],
                                      stg_v[:, 0:k_v * NP])
                o_r += k_r
                o_v += k_v

        if reps == 1:
            body()
        else:
            with tc.For_i(0, reps, 1):
                body()

    nc.compile()
    return nc


def _get_program():
    if "nc" not in _CACHE:
        _CACHE["nc"] = _build_program()
    return _CACHE["nc"]


def _centroid_perm(coords: np.ndarray):
    """Column layout: col j = c2-rank 2j, col j+512 = rank 2j+1."""
    c2_64 = (coords.astype(np.float64) ** 2).sum(1)
    order = np.argsort(c2_64, kind="stable").astype(np.int64)
    cols = np.empty(K, np.int64)
    cols[:NP] = order[0::2]
    cols[NP:] = order[1::2]
    return c2_64, order, cols


def make_in_maps(latent: np.ndarray, coords: np.ndarray) -> list[dict]:
    _, _, cols = _centroid_perm(coords)
    c2t = np.ascontiguousarray(
        (2.0 * coords[cols].T).astype(ml_dtypes.bfloat16))
    in_maps = []
    for c in range(N_CORES):
        sl = slice(c * ROWS_PER_CORE, (c + 1) * ROWS_PER_CORE)
        in_maps.append({
            "lat_t": np.ascontiguousarray(
                latent[sl].T.astype(ml_dtypes.bfloat16)),
            "c2t": c2t,
        })
    return in_maps


def kernel(latent: np.ndarray, coords: np.ndarray) -> np.ndarray:
    latent = np.asarray(latent, dtype=np.float32)
    coords = np.asarray(coords, dtype=np.float32)
    assert latent.shape == (N, D) and coords.shape == (K, D)

    nc = _get_program()
    in_maps = make_in_maps(latent, coords)
    res = run_bass_kernel_spmd(nc, in_maps, list(range(N_CORES)))

    c2_64, order, cols = _centroid_perm(coords)
    pat = _pattern()

    # Reassemble per-row score arrays. Raw tiles get u at column resolution
    # [m, 1024] (column order = cols); vred tiles get pair maxes [m, 512]
    # for pairs (rank 2j, rank 2j+1).
    raw_rows, raw_u = [], []
    pair_rows, pair_m = [], []
    for c in range(N_CORES):
        r = res.results[c]
        raws = r["gm_raw"].reshape(TILE_ROWS, -1, K)      # [p, ord, col]
        vreds = r["gm_vred"].reshape(TILE_ROWS, -1, NP)
        ords = {"R": 0, "V": 0}
        for t, mode in enumerate(pat):
            o = ords[mode]
            ords[mode] += 1
            rows = (c * ROWS_PER_CORE + t * TILE_ROWS + np.arange(TILE_ROWS))
            if mode == "R":
                raw_rows.append(rows)
                raw_u.append(raws[:, o, :])
            else:
                pair_rows.append(rows)
                pair_m.append(vreds[:, o, :])
    raw_rows = np.concatenate(raw_rows)
    raw_u = np.concatenate(raw_u).astype(np.float32)
    pair_rows = np.concatenate(pair_rows)
    pair_m = np.concatenate(pair_m).astype(np.float32)

    out = np.empty(N, np.int64)
    lat64 = latent.astype(np.float64)
    coords64 = coords.astype(np.float64)
    c2s = c2_64[order]

    # --- raw rows: per-centroid bracket ---------------------------------
    c2_cols = c2_64[cols].astype(np.float32)
    s_est = raw_u - c2_cols[None, :]
    best = s_est.max(1)
    n_cand = (s_est >= best[:, None] - MARGIN_RAW).sum(1)
    CMAXR = 4
    _resolve(out, raw_rows, s_est, n_cand, CMAXR, lat64, coords64, c2_64,
             cols.reshape(K, 1), coords64[cols].reshape(K, 1, D),
             c2_64[cols].reshape(K, 1), MARGIN_RAW)

    # --- fold/vred rows: pair bracket -----------------------------------
    c2min = c2s[0::2].astype(np.float32)
    c2max = c2s[1::2].astype(np.float32)
    ub = pair_m - c2min[None, :]
    lb = pair_m - c2max[None, :]
    best_lb = lb.max(1)
    n_cand = (ub >= best_lb[:, None] - MARGIN_PAIR).sum(1)
    CMAXP = 8
    _resolve(out, pair_rows, ub, n_cand, CMAXP, lat64, coords64, c2_64,
             order.reshape(NP, 2), coords64[order].reshape(NP, 2, D),
             c2s.reshape(NP, 2), MARGIN_PAIR)

    return out.astype(np.int32)


def _resolve(out, rows, ub, n_cand, cmax, lat64, coords64, c2, group_idx,
             group_c, group_c2, margin):
    """Resolve rows' argmin: bulk rows use top-cmax groups by ub (superset of
    candidates when n_cand <= cmax), rest fall back to the full fp64 sweep.
    First-original-index tie-breaking throughout."""
    L = group_idx.shape[1]
    bulk_m = n_cand <= cmax
    bulk = rows[bulk_m]
    if bulk.size:
        ubb = ub[bulk_m]
        gsel = np.argpartition(-ubb, cmax - 1, axis=1)[:, :cmax]
        m = bulk.size
        cands = group_c[gsel]                     # [m, C, L, D]
        sc = 2.0 * np.einsum('md,mcld->mcl', lat64[bulk], cands,
                             optimize=True) - group_c2[gsel]
        sc = sc.reshape(m, cmax * L)
        orig = group_idx[gsel].reshape(m, cmax * L)
        best = sc.max(1)
        is_best = sc >= best[:, None]
        masked = np.where(is_best, orig, np.int64(1 << 60))
        out[bulk] = masked.min(1)
    rest = rows[~bulk_m]
    if rest.size:
        sc = 2.0 * lat64[rest] @ coords64.T - c2[None, :]
        best = sc.max(1)
        is_best = sc >= best[:, None]
        masked = np.where(is_best, np.arange(len(c2))[None, :],
                          np.int64(1 << 60))
        out[rest] = masked.min(1)
